# revision 18
# baseline (speedup 1.0000x reference)
"""Trainium2 Bass kernel for nn_MicroAdder (tiny dense transformer).

Decomposition: every per-element quantity in the reference network is either
 (a) affine in the basis [u_s, w_s, 1] where u = cos(tok_angle), w = sin(tok_angle)
     (each computed with one ACT Sin op straight from int16 angle codes),
     with position-dependent constant coefficients -> ONE PE matmul per
     128-row block produces all 8 needed per-element linear forms
     (a, q0, q1, r, e0, e1, y0, y1), including the causal-softmax attention
     mixing (folded into the host-precomputed R), or
 (b) a short elementwise chain (2 rsqrt-via-ln/exp, 2 relu, ~15 two-input ops)
     on those forms, or
 (c) the final (T,V) logits expansion out = L0 (x) E0 + L1 (x) E1, done by a
     second PE matmul per block with a block-diagonal constant rhs.

v4: fp16 matmuls + burst PE transposes (fp16 PSUM, drained by DVE), ln/exp instead
of reciprocal with an activation-table selection bias so only 2 table loads
happen, two int16 angle-code planes (cos via code+pi/2 wraparound), fp16 out.

Sharding: pure data parallel over the batch dim across 8 NeuronCores.
"""

import math
import sys

import numpy as np

for _p in ("/opt/trn_rl_repo", "/root/.axon_site/_ro/trn_rl_repo"):
    if _p not in sys.path:
        sys.path.append(_p)

import concourse.bacc as bacc  # noqa: E402
import concourse.bass as bass  # noqa: E402
import concourse.tile as tile  # noqa: E402
from concourse import mybir  # noqa: E402
from concourse.bass_utils import run_bass_kernel_spmd  # noqa: E402
from concourse.masks import make_identity  # noqa: E402

# ---------------------------------------------------------------- problem dims
B, T, V = 65536, 34, 14
D, EPS, MAX_DIGITS = 5, 1e-5, 10
NCORES = 8
BC = B // NCORES            # rows per core = 8192
P = 128                     # partitions
NPER = BC // P              # rows per partition = 64
NBLK = NPER                 # blocks per core = 64 (block j = rows {p*NPER + j})
K1 = 2 * T + 1              # basis size = 69
NG = 8                      # matmul1 groups
N1 = NG * T                 # 272
N2 = T * V                  # 476
NPRM = 8
NCHUNK = 8                  # phase-D chunks
CB = NBLK // NCHUNK         # blocks per chunk = 16
OGRP = 8                    # blocks per output DMA

F32 = mybir.dt.float32
F16 = mybir.dt.float16
I16 = mybir.dt.int16
AF = mybir.ActivationFunctionType
ALU = mybir.AluOpType

# group order in matmul1 output columns (g*T..g*T+T)
# [a | q0 q1 | r e0 e1 y0 y1]: r+e+y drain in one copy; ar = PSUM(a) x SBUF(r)
G_A, G_Q0, G_Q1, G_R, G_E0, G_E1, G_Y0, G_Y1 = range(8)

# PRM slots
(P_RAT, P_SQ0, P_C3, P_H00, P_H10, P_H01, P_H11, P_SPARE) = range(8)
HS2 = 2.0 * math.pi / 65536.0  # angle units per int16 code step (mod 2pi wrap)


def _patch_act_tables():
    """Bias the act-table selection so Exp/Ln/Relu/Copy/Square all resolve to
    natural_log_exp_and_others (one table) instead of three different ones.
    The scalar engine holds ~2 resident tables; spanning 3 causes a ~1.5us
    table reload per ln/exp pair. Only the selection is biased -- emitted
    act_func_set_ids still index the unmodified act_info.json."""
    if getattr(bacc, "_microadder_act_patch", False):
        return
    from concourse.hw_specs import get_activation_tables as orig

    strip = {AF.Exp, AF.Ln, AF.Relu, AF.Copy, AF.Square, AF.Identity}

    def patched(arch):
        tabs = orig(arch)
        out = {}
        before = True
        for name, s in tabs.items():
            if name == "natural_log_exp_and_others":
                before = False
            out[name] = (s - strip) if before else s
        return out

    bacc.get_activation_tables = patched
    bacc._microadder_act_patch = True


# ---------------------------------------------------------------- host tables
def host_tables(tok_A, tok_start, tok_stride, sp_amp, sp_phase, sp_slope, sp_offset,
                norm_w, q_w, q_phase, out_A, out_B, fc1_w, fc2_w, head_w):
    f = np.float64
    A = f(tok_A)
    t = np.arange(T, dtype=f)
    th = 2.0 * np.pi * t / MAX_DIGITS + f(sp_phase)
    pos = np.stack([f(sp_amp) * np.cos(th), f(sp_amp) * np.sin(th),
                    f(sp_slope) * t + f(sp_offset)], axis=-1)
    k = pos @ np.asarray(q_w, f).T
    c0, s0 = np.cos(f(q_phase[0])), np.sin(f(q_phase[0]))
    q = k.copy()
    q[:, 0] = c0 * k[:, 0] - s0 * k[:, 1]
    q[:, 1] = s0 * k[:, 0] + c0 * k[:, 1]
    scores = (q @ k.T) / np.sqrt(f(5.0))
    sm = np.where(np.tril(np.ones((T, T), bool)), scores, -np.inf)
    sm = sm - sm.max(-1, keepdims=True)
    e = np.exp(sm)
    attn = e / e.sum(-1, keepdims=True)

    nw = np.asarray(norm_w, f)
    oA = np.asarray(out_A, f)[:, 0]
    oB = np.asarray(out_B, f)[0]
    S_t = A * A + (pos ** 2).sum(-1)
    rms1 = np.sqrt(S_t / D + EPS)

    M0 = attn * (A * nw[0] * oA[0] / rms1)[None, :]
    M1 = attn * (A * nw[1] * oA[1] / rms1)[None, :]
    c_t = attn @ ((pos * (nw[2:] * oA[2:])[None, :]).sum(-1) / rms1)

    g0 = np.asarray(fc2_w, f)[:, 0]
    g1 = np.asarray(fc2_w, f)[:, 1]
    projs = {
        G_Q0: nw * np.asarray(fc1_w, f)[0],
        G_Q1: nw * np.asarray(fc1_w, f)[1],
        G_E0: 2.0 * g0,
        G_E1: 2.0 * g1,
        G_Y0: nw * np.asarray(head_w, f)[0],
        G_Y1: nw * np.asarray(head_w, f)[1],
    }
    R = np.zeros((K1, NG * T), dtype=f)
    dd = np.eye(T, dtype=f)
    for gi in range(NG):
        cols = slice(gi * T, (gi + 1) * T)
        if gi == G_A:
            R[0:T, cols] = M0.T
            R[T:2 * T, cols] = M1.T
            R[2 * T, cols] = c_t
        elif gi == G_R:
            b2 = (oB ** 2).sum()
            R[0:T, cols] = 2 * A * oB[0] * dd + b2 * M0.T
            R[T:2 * T, cols] = 2 * A * oB[1] * dd + b2 * M1.T
            R[2 * T, cols] = 2 * (pos * oB[None, 2:]).sum(-1) + b2 * c_t
        else:
            v = projs[gi]
            bv = (oB * v).sum()
            R[0:T, cols] = A * v[0] * dd + bv * M0.T
            R[T:2 * T, cols] = A * v[1] * dd + bv * M1.T
            R[2 * T, cols] = (pos * v[None, 2:]).sum(-1) + bv * c_t

    G00, G01, G11 = (g0 * g0).sum(), (g0 * g1).sum(), (g1 * g1).sum()
    if G00 > 1e-30:
        sq0, rat = np.sqrt(G00), G01 / G00
        c3 = np.sqrt(max(G11 - G01 * G01 / G00, 0.0))
    else:
        sq0, rat, c3 = 0.0, 0.0, np.sqrt(G11)
    hv0 = nw * np.asarray(head_w, f)[0]
    hv1 = nw * np.asarray(head_w, f)[1]
    H = np.array([[(g0 * hv0).sum(), (g0 * hv1).sum()],
                  [(g1 * hv0).sum(), (g1 * hv1).sum()]])

    dvoc = np.arange(V, dtype=f)
    ang = f(tok_start) + dvoc * f(tok_stride)
    E = np.stack([A * np.cos(ang), A * np.sin(ang)], axis=-1)
    RHS2 = np.zeros((2 * T, N2), dtype=f)
    for t_ in range(T):
        RHS2[t_, t_ * V:(t_ + 1) * V] = E[:, 0]
        RHS2[T + t_, t_ * V:(t_ + 1) * V] = E[:, 1]

    # idx -> two int16 angle-code planes: c encodes ang (mod 2pi over the full
    # int16 wrap), c2 encodes ang + pi/2, so  w = Sin(HS2*c), u = Sin(HS2*c2)
    # both with LUT args in [-pi, pi).
    angv = np.mod(f(tok_start) + np.arange(V, dtype=f) * f(tok_stride), 2 * np.pi)
    c_w = np.round(angv / (2 * np.pi) * 65536.0).astype(np.int64)
    c_u = c_w + 16384
    CODE2 = np.stack([(c_u & 0xFFFF).astype(np.uint16).astype(np.int16),
                      (c_w & 0xFFFF).astype(np.uint16).astype(np.int16)],
                     axis=-1)  # [V, 2] -> (u-code, w-code)

    SROW = np.tile(S_t, NBLK)[None, :]  # [1, 64*34]
    PRM = np.zeros((1, NPRM), dtype=np.float64)
    PRM[0, P_RAT] = rat
    PRM[0, P_SQ0] = sq0
    PRM[0, P_C3] = c3
    PRM[0, P_H00] = H[0, 0]
    PRM[0, P_H10] = H[1, 0]
    PRM[0, P_H01] = H[0, 1]
    PRM[0, P_H11] = H[1, 1]
    Rp = np.zeros((P, NG * T))
    Rp[:K1] = R
    RHS2p = np.zeros((P, N2))
    RHS2p[:2 * T] = RHS2
    return (Rp.astype(np.float32), RHS2p.astype(np.float32),
            np.ascontiguousarray(SROW, np.float32).copy(),
            np.ascontiguousarray(PRM, np.float32).copy(), CODE2)


# ---------------------------------------------------------------- bass kernel
def build_bass():
    _patch_act_tables()
    nc = bacc.Bacc("TRN2", target_bir_lowering=False, debug=False)

    idx_d = nc.dram_tensor("idx", [BC, 2 * T], I16, kind="ExternalInput").ap()
    r_d = nc.dram_tensor("R", [P, N1], F32, kind="ExternalInput").ap()
    rhs2_d = nc.dram_tensor("RHS2", [P, N2], F32, kind="ExternalInput").ap()
    srow_d = nc.dram_tensor("SROW", [1, NBLK * T], F32, kind="ExternalInput").ap()
    prm_d = nc.dram_tensor("PRM", [1, NPRM], F32, kind="ExternalInput").ap()
    out_d = nc.dram_tensor("out", [BC, N2], F16, kind="ExternalOutput").ap()

    # DRAM views: partition p holds rows p*NPER .. p*NPER+NPER-1
    idx_v = idx_d.rearrange("(p n) (two t) -> p n two t", p=P, two=2)
    out_v = out_d.rearrange("(p n) c -> p n c", p=P)       # [128, 64, 476]

    with tile.TileContext(nc) as tc:
        with (
            tc.tile_pool(name="const", bufs=1) as cpool,
            tc.tile_pool(name="uwtp", bufs=2) as uwtp,
            tc.tile_pool(name="ptp", bufs=2) as ptp,
            tc.tile_pool(name="dr", bufs=3) as drp,
            tc.tile_pool(name="ph", bufs=3) as php,
            tc.tile_pool(name="outsb", bufs=3) as outp,
            tc.tile_pool(name="pm1", bufs=1, space="PSUM") as pm1p,
            tc.tile_pool(name="pm2", bufs=2, space="PSUM") as pm2p,
            tc.tile_pool(name="ptr", bufs=2, space="PSUM") as ptrp,
        ):
            # ---- constants / inputs
            idx_sb = cpool.tile([P, NBLK, 2, T], I16)
            nc.sync.dma_start(idx_sb[:], idx_v)
            ident = cpool.tile([P, P], F16)
            make_identity(nc, ident[:])
            r32 = cpool.tile([P, N1], F32)
            nc.sync.dma_start(r32[:], r_d)
            r_f16 = cpool.tile([P, N1], F16)
            nc.vector.tensor_copy(r_f16[:], r32[:])
            rhs32 = cpool.tile([P, N2], F32)
            nc.sync.dma_start(rhs32[:], rhs2_d)
            rhs2_f16 = cpool.tile([P, N2], F16)
            nc.vector.tensor_copy(rhs2_f16[:], rhs32[:])
            # manual double-buffered K=128-padded stationary tiles (FWL):
            # pad rows (K1:128 / 2T:128) zeroed once, never rewritten
            uwT_bufs = [cpool.tile([P, 8, P], F16, name=f"uwTb{_i}")
                        for _i in range(2)]
            pT_bufs = [cpool.tile([P, 8, P], F16, name=f"pTb{_i}")
                       for _i in range(2)]
            for _tb in uwT_bufs + pT_bufs:
                nc.vector.memset(_tb[64:P, :, :], 0.0)
            s32 = cpool.tile([P, NBLK * T], F32)
            nc.sync.dma_start(s32[:], srow_d.broadcast_to([P, NBLK * T]))
            s_f16 = cpool.tile([P, NBLK, T], F16)
            nc.vector.tensor_copy(s_f16[:], s32[:].rearrange("p (n t) -> p n t", t=T))
            prm_sb = cpool.tile([P, NPRM], F32)
            nc.sync.dma_start(prm_sb[:], prm_d.broadcast_to([P, NPRM]))

            def prm(i):
                return prm_sb[:, i:i + 1]

            # f32 [128,1] constant APs for activation scale/bias operands
            cns_t = cpool.tile([P, 3], F32)
            for _i, _v in enumerate([HS2, 1.0 / D, -0.5]):
                nc.vector.memset(cns_t[:, _i:_i + 1], _v)
            C_HS2, C_ID, C_NHALF = (cns_t[:, _i:_i + 1] for _i in range(3))
            C_EPS = cns_t[:, 2:3]  # placeholder, replaced below
            cns2_t = cpool.tile([P, 1], F32)
            nc.vector.memset(cns2_t[:], EPS)
            C_EPS = cns2_t[:]

            # ---- phase A (whole core): codes -> [u, w, 1] basis, fp16
            uw = cpool.tile([P, NBLK, K1], F16)   # [u(34) w(34) 1]
            nc.scalar.activation(uw[:, :, 0:T], idx_sb[:, :, 0, :], AF.Sin,
                                 bias=0.0, scale=C_HS2)
            nc.scalar.activation(uw[:, :, T:2 * T], idx_sb[:, :, 1, :], AF.Sin,
                                 bias=0.0, scale=C_HS2)
            nc.vector.memset(uw[:, :, 2 * T:K1], 1.0)

            # p tile (lint forms), written by phase D, transposed for matmul2
            p_t = cpool.tile([P, NBLK, 2 * T], F16)

            state = {}

            def emit_A(ch):
                """matmul1 stage: uw transposes (PE), K-padded stationary
                copies (DVE), matmul1 (PE), form drains (ACT relu + DVE)."""
                j0 = ch * CB
                ar_c = drp.tile([P, CB, T], F16, tag="ar")
                rho_c = drp.tile([P, CB, 2 * T], F16, tag="rho")
                tey_c = drp.tile([P, CB, 5 * T], F16, tag="tey")
                state[ch] = (ar_c, rho_c, tey_c)
                for o in range(CB // 8):
                    jo = j0 + 8 * o
                    ptr1 = ptrp.tile([K1, 8, P], F16, tag="ptr")
                    for b in range(8):
                        nc.tensor.transpose(ptr1[:, b, :], uw[:, jo + b, :],
                                            ident[:])
                    uwT = uwT_bufs[o % 2]
                    nc.vector.tensor_copy(uwT[0:K1, :, :], ptr1[:])
                    for q2 in range(2):
                        jq = 8 * o + 4 * q2
                        pm1 = pm1p.tile([P, 4, 512], F32, tag="pm1")
                        for b in range(4):
                            nc.tensor.matmul(pm1[:, b, 0:N1],
                                             uwT[:, 4 * q2 + b, :], r_f16[:],
                                             start=True, stop=True)
                        qsl = slice(jq, jq + 4)
                        # drains: rho = relu(q01) (ACT), r/e/y copy (DVE),
                        # then ar = PSUM(a) * SBUF(r) -- one PSUM input max
                        nc.scalar.activation(rho_c[:, qsl, :],
                                             pm1[:, :, T:3 * T],
                                             AF.Relu, bias=0.0, scale=1.0)
                        nc.vector.tensor_copy(tey_c[:, qsl, :],
                                              pm1[:, :, 3 * T:8 * T])
                        nc.vector.tensor_mul(
                            ar_c[:, qsl, :], pm1[:, :, 0:T],
                            tey_c[:, qsl, 0:T])

            def emit_B(ch):
                """phase D: chunk-wide elementwise chain (free = CB*T = 544)."""
                j0 = ch * CB
                csl = slice(j0, j0 + CB)
                ar_c, rho_c, tey_c = state[ch]
                rho0 = rho_c[:, :, 0:T]
                rho1 = rho_c[:, :, T:2 * T]
                e0 = tey_c[:, :, T:2 * T]
                e1 = tey_c[:, :, 2 * T:3 * T]
                y0 = tey_c[:, :, 3 * T:4 * T]
                y1 = tey_c[:, :, 4 * T:5 * T]

                n2 = php.tile([P, CB, T], F16, tag="n2")
                nc.vector.tensor_add(n2[:], ar_c[:], s_f16[:, csl, :])
                inv2 = php.tile([P, CB, T], F16, tag="inv2")
                nc.scalar.activation(inv2[:], n2[:], AF.Ln, bias=C_EPS,
                                     scale=C_ID)
                nc.scalar.activation(inv2[:], inv2[:], AF.Exp, bias=0.0,
                                     scale=C_NHALF)
                z0 = php.tile([P, CB, T], F16, tag="z0")
                nc.vector.tensor_mul(z0[:], rho0, inv2[:])
                z1 = php.tile([P, CB, T], F16, tag="z1")
                nc.vector.tensor_mul(z1[:], rho1, inv2[:])
                v1 = php.tile([P, CB, T], F16, tag="v1")
                nc.vector.scalar_tensor_tensor(v1[:], z1[:], prm(P_RAT), z0[:],
                                               op0=ALU.mult, op1=ALU.add)
                v1v1 = php.tile([P, CB, T], F16, tag="v1v1")
                nc.scalar.activation(v1v1[:], v1[:], AF.Square, bias=0.0,
                                     scale=prm(P_SQ0))
                z1z1 = php.tile([P, CB, T], F16, tag="z1z1")
                nc.scalar.activation(z1z1[:], z1[:], AF.Square, bias=0.0,
                                     scale=prm(P_C3))
                t0 = php.tile([P, CB, T], F16, tag="t0")
                nc.vector.tensor_mul(t0[:], z0[:], e0)
                t1 = php.tile([P, CB, T], F16, tag="t1")
                nc.gpsimd.tensor_mul(t1[:], z1[:], e1)
                n3 = php.tile([P, CB, T], F16, tag="n3")
                nc.vector.tensor_add(n3[:], v1v1[:], n2[:])
                nc.vector.tensor_add(n3[:], z1z1[:], n3[:])
                nc.vector.tensor_add(n3[:], t0[:], n3[:])
                nc.vector.tensor_add(n3[:], t1[:], n3[:])
                inv3 = php.tile([P, CB, T], F16, tag="inv3")
                nc.scalar.activation(inv3[:], n3[:], AF.Ln, bias=C_EPS,
                                     scale=C_ID)
                nc.scalar.activation(inv3[:], inv3[:], AF.Exp, bias=0.0,
                                     scale=C_NHALF)

                p0 = php.tile([P, CB, T], F16, tag="p0")
                nc.vector.scalar_tensor_tensor(p0[:], z0[:], prm(P_H00), y0,
                                               op0=ALU.mult, op1=ALU.add)
                nc.vector.scalar_tensor_tensor(p0[:], z1[:], prm(P_H10), p0[:],
                                               op0=ALU.mult, op1=ALU.add)
                nc.vector.tensor_mul(p_t[:, csl, 0:T], p0[:], inv3[:])
                p1 = php.tile([P, CB, T], F16, tag="p1")
                nc.vector.scalar_tensor_tensor(p1[:], z0[:], prm(P_H01), y1,
                                               op0=ALU.mult, op1=ALU.add)
                nc.vector.scalar_tensor_tensor(p1[:], z1[:], prm(P_H11), p1[:],
                                               op0=ALU.mult, op1=ALU.add)
                nc.vector.tensor_mul(p_t[:, csl, T:2 * T], p1[:], inv3[:])

            def emit_C(ch):
                """matmul2 stage: p transposes (PE), K-padded stationary copies
                (DVE), matmul2 (PE), out copies (DVE/ACT), store DMA."""
                j0 = ch * CB
                for o in range(CB // 8):
                    jo = j0 + 8 * o
                    ptr2 = ptrp.tile([K1, 8, P], F16, tag="ptr")
                    for b in range(8):
                        nc.tensor.transpose(ptr2[0:2 * T, b, :],
                                            p_t[:, jo + b, :], ident[:])
                    pT = pT_bufs[o % 2]
                    nc.vector.tensor_copy(pT[0:2 * T, :, :],
                                          ptr2[0:2 * T, :, :])
                    o_sb = outp.tile([P, OGRP, N2], F16, tag="osb")
                    for b in range(8):
                        j = jo + b
                        pm2 = pm2p.tile([P, 512], F32, tag="pm2")
                        nc.tensor.matmul(pm2[:, 0:N2], pT[:, b, :],
                                         rhs2_f16[:], start=True, stop=True)
                        if j % 16 == 0:
                            nc.vector.tensor_copy(o_sb[:, b, :], pm2[:, 0:N2])
                        else:
                            nc.scalar.copy(o_sb[:, b, :], pm2[:, 0:N2])
                    nc.sync.dma_start(out_v[:, jo:jo + OGRP, :], o_sb[:])

            # software pipeline: A(ch+1) is emitted before C(ch) so the PE's
            # in-order queue never head-of-line blocks on phase D
            emit_A(0)
            emit_B(0)
            for ch in range(1, NCHUNK):
                emit_A(ch)
                emit_C(ch - 1)
                emit_B(ch)
            emit_C(NCHUNK - 1)

    nc.compile()
    return nc


_CACHE = {}


def _get_nc():
    if "nc" not in _CACHE:
        _CACHE["nc"] = build_bass()
    return _CACHE["nc"]


def make_in_maps(inputs):
    idx = np.asarray(inputs["idx"]).astype(np.int32)
    kw = {k: np.asarray(v, np.float64) for k, v in inputs.items() if k != "idx"}
    R, RHS2, SROW, PRM, CODE2 = host_tables(**kw)
    idxc = CODE2[idx]                        # [B, T, 2]
    idxc = np.ascontiguousarray(
        idxc.transpose(0, 2, 1).reshape(B, 2 * T))  # [B, (2,T)] int16
    return [
        {"idx": idxc[c * BC:(c + 1) * BC], "R": R, "RHS2": RHS2,
         "SROW": SROW, "PRM": PRM}
        for c in range(NCORES)
    ]


def kernel(**inputs) -> np.ndarray:
    nc = _get_nc()
    in_maps = make_in_maps(inputs)
    res = run_bass_kernel_spmd(nc, in_maps, core_ids=list(range(NCORES)))
    out = np.concatenate(
        [np.asarray(res.results[c]["out"]).astype(np.float32)
         for c in range(NCORES)], axis=0)
    return np.ascontiguousarray(out.reshape(B, T, V))
